# revision 7
# baseline (speedup 1.0000x reference)
"""Trainium2 Bass kernel for nn_Attention_47459388621522.

Computation (B=256, N=2048, D=256):
    hidden = concat([feature, broadcast(pointer_hidden_state)], -1)   # [B,N,2D]
    pre    = tanh(einsum('de,bne->bnd', W[0], hidden))                # [B,N,D]
    scores = einsum('d,bnd->bn', v[0,0], pre)                         # [B,N]
    attns  = softmax(scores, axis=1)[:, None, :]                      # [B,1,N]

Split W = [Wf | Wh] along e: pre = tanh(feature @ Wf^T + bias_b) with
bias = pointer_hidden_state @ Wh^T precomputed per batch on-device (tiny).

Sharding: data-parallel over batch, 32 batches per core x 8 cores.
Host pre-transposes feature to [32, D, N] per core so the contraction dim e
lands on SBUF partitions with fully-contiguous DMA rows.

Per-core dataflow (tokens grouped 512 at a time, d in 2 chunks of 128):
    PE : pre[d,t]   = WfT[e,d]^T @ featT[e,t]      (2 e-chunk accumulating MMs)
    ACT: th[d,t]    = tanh(pre + bias[d,b])        (per-partition bias, PSUM->SBUF)
    PE : sc[1,t]    = v[d,1]^T @ th[d,t]           (2 d-chunk accumulating MMs)
    DMA: sc -> scores_all[b, t]
    softmax over [32, 2048] at the end (DVE reduce-max, ACT exp+accum, DVE mul)
"""

import numpy as np

import concourse.bacc as bacc
import concourse.mybir as mybir
import concourse.tile as tile
from concourse.bass_utils import run_bass_kernel_spmd

f32 = mybir.dt.float32
f32r = mybir.dt.float32r

B, N, D = 256, 2048, 256
N_CORES = 8
B_PER = B // N_CORES          # 32 batches per core
TG = 512                      # token group (matmul moving free dim)
NG = N // TG                  # 4 groups per batch
P = 128
DC = D // P                   # 2 d-chunks
KC = D // P                   # 2 e-chunks

# main matmul dtype: f32r = 1 cyc/row (inputs rounded to 12 mantissa bits),
# f32 = exact but 4 cyc/row.
MM_DT = f32r

_CACHED = {}


def _build():
    nc = bacc.Bacc("TRN2", target_bir_lowering=False, debug=False, name="ptrattn")
    featT = nc.dram_tensor("featT", [B_PER, D, N], f32, kind="ExternalInput")
    hT = nc.dram_tensor("hT", [D, B_PER], f32, kind="ExternalInput")
    wfT = nc.dram_tensor("wfT", [D, D], f32, kind="ExternalInput")
    whT = nc.dram_tensor("whT", [D, D], f32, kind="ExternalInput")
    vv = nc.dram_tensor("vv", [D, 1], f32, kind="ExternalInput")
    out = nc.dram_tensor("attns", [B_PER, N], f32, kind="ExternalOutput")

    act = mybir.ActivationFunctionType

    with tile.TileContext(nc) as tc:
        with tc.tile_pool(name="singles", bufs=1) as singles, \
             tc.tile_pool(name="feat", bufs=2) as feat_pool, \
             tc.tile_pool(name="th", bufs=3) as th_pool, \
             tc.tile_pool(name="stage", bufs=3) as stage_pool, \
             tc.tile_pool(name="soft", bufs=1) as soft_pool, \
             tc.tile_pool(name="mmps", bufs=2, space="PSUM") as mmps, \
             tc.tile_pool(name="scps", bufs=3, space="PSUM") as scps:

            # ---- constants ----
            wf_sb = singles.tile([P, KC, D], MM_DT)
            nc.sync.dma_start(
                wf_sb, wfT.rearrange("(ko p) d -> p ko d", p=P).bitcast(MM_DT))
            # Wh^T laid out [e, d]; bias needs lhsT=[e,d] chunks + rhs hT [e,b]
            wh_full = singles.tile([P, KC, D], f32)
            nc.sync.dma_start(wh_full, whT.rearrange("(ko p) d -> p ko d", p=P))
            hT_sb = singles.tile([P, KC, B_PER], f32)
            nc.sync.dma_start(hT_sb, hT.rearrange("(ko p) b -> p ko b", p=P))
            v_sb = singles.tile([P, DC, 1], f32r)
            nc.sync.dma_start(
                v_sb, vv.rearrange("(ko p) one -> p ko one", p=P).bitcast(f32r))

            # ---- bias[b, d] = Wh @ h_b  (exact fp32, tiny) ----
            bias_sb = singles.tile([P, DC, B_PER], f32)
            for dc in range(DC):
                bias_ps = mmps.tile([P, B_PER], f32, tag=f"pre{dc}")
                for ko in range(KC):
                    nc.tensor.matmul(
                        bias_ps,
                        wh_full[:, ko, dc * P:(dc + 1) * P],
                        hT_sb[:, ko, :],
                        start=(ko == 0), stop=(ko == KC - 1),
                    )
                nc.vector.tensor_copy(bias_sb[:, dc, :], bias_ps)

            # scores accumulator [batch, N] on partitions 0..B_PER-1
            scores_all = soft_pool.tile([B_PER, N], f32)

            # ---- main loop ----
            for b in range(B_PER):
                ft = feat_pool.tile([P, KC, N], MM_DT, tag="ft")
                nc.sync.dma_start(
                    ft, featT[b].rearrange("(ko p) n -> p ko n", p=P).bitcast(MM_DT))

                stage = stage_pool.tile([1, N], f32, tag="stage")
                for g in range(NG):
                    ts = slice(g * TG, (g + 1) * TG)
                    th = th_pool.tile([P, DC, TG], f32r, tag="th")
                    for dc in range(DC):
                        pre = mmps.tile([P, TG], f32, tag=f"pre{dc}")
                        for ko in range(KC):
                            nc.tensor.matmul(
                                pre,
                                wf_sb[:, ko, dc * P:(dc + 1) * P],
                                ft[:, ko, ts],
                                start=(ko == 0), stop=(ko == KC - 1),
                            )
                        nc.scalar.activation(
                            th[:, dc, :], pre, act.Tanh,
                            bias=bias_sb[:, dc, b:b + 1], scale=1.0)
                    sc = scps.tile([1, TG], f32, tag="sc")
                    for dc in range(DC):
                        nc.tensor.matmul(
                            sc, v_sb[:, dc, :], th[:, dc, :],
                            start=(dc == 0), stop=(dc == DC - 1),
                        )
                    nc.vector.tensor_copy(stage[:, ts], sc)
                nc.sync.dma_start(scores_all[b:b + 1, :], stage)

            # ---- softmax over N per batch row ----
            negmax = soft_pool.tile([B_PER, 1], f32)
            nc.vector.tensor_reduce(
                negmax, scores_all, axis=mybir.AxisListType.X,
                op=mybir.AluOpType.max, negate=True)
            shifted = soft_pool.tile([B_PER, N], f32)
            nc.vector.tensor_scalar(
                shifted, scores_all, negmax, -80.0,
                op0=mybir.AluOpType.add, op1=mybir.AluOpType.max)
            probs = soft_pool.tile([B_PER, N], f32)
            sumexp = soft_pool.tile([B_PER, 1], f32)
            nc.scalar.activation(
                probs, shifted, act.Exp, bias=0.0, scale=1.0, accum_out=sumexp)
            rcp = soft_pool.tile([B_PER, 1], f32)
            nc.vector.reciprocal(rcp, sumexp)
            nc.vector.tensor_scalar_mul(probs, probs, rcp)
            nc.sync.dma_start(out.ap(), probs)

    nc.compile()
    return nc


def kernel(feature, pointer_hidden_state, v, W):
    feature = np.ascontiguousarray(feature, dtype=np.float32)
    pointer_hidden_state = np.ascontiguousarray(pointer_hidden_state, dtype=np.float32)
    v = np.asarray(v, dtype=np.float32)
    W = np.asarray(W, dtype=np.float32)

    wfT = np.ascontiguousarray(W[0][:, :D].T)   # [e, d]
    whT = np.ascontiguousarray(W[0][:, D:].T)   # [e, d]
    vv = np.ascontiguousarray(v[0, 0][:, None])  # [D, 1]

    if "nc" not in _CACHED:
        _CACHED["nc"] = _build()
    nc = _CACHED["nc"]

    in_maps = []
    for c in range(N_CORES):
        sl = slice(c * B_PER, (c + 1) * B_PER)
        featT = np.ascontiguousarray(feature[sl].transpose(0, 2, 1))  # [B_PER, D, N]
        hT = np.ascontiguousarray(pointer_hidden_state[sl].T)         # [D, B_PER]
        in_maps.append({"featT": featT, "hT": hT, "wfT": wfT, "whT": whT, "vv": vv})

    res = run_bass_kernel_spmd(nc, in_maps, core_ids=list(range(N_CORES)))
    _CACHED["last_res"] = res
    outs = [res.results[c]["attns"] for c in range(N_CORES)]
    return np.concatenate(outs, axis=0)[:, None, :].astype(np.float32)


# revision 10
# speedup vs baseline: 100.6802x; 100.6802x over previous
"""Trainium2 Bass kernel for nn_Attention_47459388621522.

Computation (B=256, N=2048, D=256):
    hidden = concat([feature, broadcast(pointer_hidden_state)], -1)   # [B,N,2D]
    pre    = tanh(einsum('de,bne->bnd', W[0], hidden))                # [B,N,D]
    scores = einsum('d,bnd->bn', v[0,0], pre)                         # [B,N]
    attns  = softmax(scores, axis=1)[:, None, :]                      # [B,1,N]

Split W = [Wf | Wh] along e: pre = tanh(feature @ Wf^T + bias_b) with
bias = pointer_hidden_state @ Wh^T precomputed per batch on-device (tiny).

Sharding: data-parallel over batch, 32 batches per core x 8 cores.
Host pre-transposes feature to [32, D, N] per core so the contraction dim e
lands on SBUF partitions with fully-contiguous DMA rows.

Per-core dataflow (tokens grouped 512 at a time, d in 2 chunks of 128):
    PE : pre[d,t]   = WfT[e,d]^T @ featT[e,t]      (2 e-chunk accumulating MMs)
    ACT: th[d,t]    = tanh(pre + bias[d,b])        (per-partition bias, PSUM->SBUF)
    PE : sc[1,t]    = v[d,1]^T @ th[d,t]           (2 d-chunk accumulating MMs)
    DMA: sc -> scores_all[b, t]
    softmax over [32, 2048] at the end (DVE reduce-max, ACT exp+accum, DVE mul)
"""

import numpy as np

import concourse.bacc as bacc
import concourse.mybir as mybir
import concourse.tile as tile
from concourse.bass_utils import run_bass_kernel_spmd

f32 = mybir.dt.float32
f32r = mybir.dt.float32r

B, N, D = 256, 2048, 256
N_CORES = 8
B_PER = B // N_CORES          # 32 batches per core
TG = 512                      # token group (matmul moving free dim)
NG = N // TG                  # 4 groups per batch
P = 128
DC = D // P                   # 2 d-chunks
KC = D // P                   # 2 e-chunks

# main matmul dtype: f32r = 1 cyc/row (inputs rounded to 12 mantissa bits),
# f32 = exact but 4 cyc/row.
MM_DT = f32r

_CACHED = {}


def _repeat_range(repeat):
    # repeat>1 builds a kernel that does the batch loop `repeat` times —
    # used only by the timing harness to separate device time from
    # per-dispatch overhead via slope fitting.
    for _ in range(repeat):
        yield from range(B_PER)


def _build(repeat=1):
    nc = bacc.Bacc("TRN2", target_bir_lowering=False, debug=False, name="ptrattn")
    featT = nc.dram_tensor("featT", [B_PER, D, N], f32, kind="ExternalInput")
    hT = nc.dram_tensor("hT", [D, B_PER], f32, kind="ExternalInput")
    wfT = nc.dram_tensor("wfT", [D, D], f32, kind="ExternalInput")
    whT = nc.dram_tensor("whT", [D, D], f32, kind="ExternalInput")
    vv = nc.dram_tensor("vv", [D, 1], f32, kind="ExternalInput")
    out = nc.dram_tensor("attns", [B_PER, N], f32, kind="ExternalOutput")

    act = mybir.ActivationFunctionType

    with tile.TileContext(nc) as tc:
        with tc.tile_pool(name="singles", bufs=1) as singles, \
             tc.tile_pool(name="feat", bufs=2) as feat_pool, \
             tc.tile_pool(name="th", bufs=3) as th_pool, \
             tc.tile_pool(name="stage", bufs=3) as stage_pool, \
             tc.tile_pool(name="soft", bufs=1) as soft_pool, \
             tc.tile_pool(name="mmps", bufs=2, space="PSUM") as mmps, \
             tc.tile_pool(name="scps", bufs=3, space="PSUM") as scps:

            # ---- constants ----
            wf_sb = singles.tile([P, KC, D], MM_DT)
            nc.sync.dma_start(
                wf_sb, wfT.rearrange("(ko p) d -> p ko d", p=P).bitcast(MM_DT))
            # Wh^T laid out [e, d]; bias needs lhsT=[e,d] chunks + rhs hT [e,b]
            wh_full = singles.tile([P, KC, D], f32)
            nc.sync.dma_start(wh_full, whT.rearrange("(ko p) d -> p ko d", p=P))
            hT_sb = singles.tile([P, KC, B_PER], f32)
            nc.sync.dma_start(hT_sb, hT.rearrange("(ko p) b -> p ko b", p=P))
            v_sb = singles.tile([P, DC, 1], f32r)
            nc.sync.dma_start(
                v_sb, vv.rearrange("(ko p) one -> p ko one", p=P).bitcast(f32r))

            # ---- bias[b, d] = Wh @ h_b  (exact fp32, tiny) ----
            bias_sb = singles.tile([P, DC, B_PER], f32)
            for dc in range(DC):
                bias_ps = mmps.tile([P, B_PER], f32, tag=f"pre{dc}")
                for ko in range(KC):
                    nc.tensor.matmul(
                        bias_ps,
                        wh_full[:, ko, dc * P:(dc + 1) * P],
                        hT_sb[:, ko, :],
                        start=(ko == 0), stop=(ko == KC - 1),
                    )
                nc.vector.tensor_copy(bias_sb[:, dc, :], bias_ps)

            # scores accumulator [batch, N] on partitions 0..B_PER-1
            scores_all = soft_pool.tile([B_PER, N], f32)

            # ---- main loop ----
            for b in _repeat_range(repeat):
                ft = feat_pool.tile([P, KC, N], MM_DT, tag="ft")
                nc.sync.dma_start(
                    ft, featT[b].rearrange("(ko p) n -> p ko n", p=P).bitcast(MM_DT))

                stage = stage_pool.tile([1, N], f32, tag="stage")
                for g in range(NG):
                    ts = slice(g * TG, (g + 1) * TG)
                    th = th_pool.tile([P, DC, TG], f32r, tag="th")
                    for dc in range(DC):
                        pre = mmps.tile([P, TG], f32, tag=f"pre{dc}")
                        for ko in range(KC):
                            nc.tensor.matmul(
                                pre,
                                wf_sb[:, ko, dc * P:(dc + 1) * P],
                                ft[:, ko, ts],
                                start=(ko == 0), stop=(ko == KC - 1),
                            )
                        nc.scalar.activation(
                            th[:, dc, :], pre, act.Tanh,
                            bias=bias_sb[:, dc, b:b + 1], scale=1.0)
                    sc = scps.tile([1, TG], f32, tag="sc")
                    for dc in range(DC):
                        nc.tensor.matmul(
                            sc, v_sb[:, dc, :], th[:, dc, :],
                            start=(dc == 0), stop=(dc == DC - 1),
                        )
                    nc.vector.tensor_copy(stage[:, ts], sc)
                nc.sync.dma_start(scores_all[b:b + 1, :], stage)

            # ---- softmax over N per batch row ----
            negmax = soft_pool.tile([B_PER, 1], f32)
            nc.vector.tensor_reduce(
                negmax, scores_all, axis=mybir.AxisListType.X,
                op=mybir.AluOpType.max, negate=True)
            shifted = soft_pool.tile([B_PER, N], f32)
            nc.vector.tensor_scalar(
                shifted, scores_all, negmax, -80.0,
                op0=mybir.AluOpType.add, op1=mybir.AluOpType.max)
            probs = soft_pool.tile([B_PER, N], f32)
            sumexp = soft_pool.tile([B_PER, 1], f32)
            nc.scalar.activation(
                probs, shifted, act.Exp, bias=0.0, scale=1.0, accum_out=sumexp)
            rcp = soft_pool.tile([B_PER, 1], f32)
            nc.vector.reciprocal(rcp, sumexp)
            nc.vector.tensor_scalar_mul(probs, probs, rcp)
            nc.sync.dma_start(out.ap(), probs)

    nc.compile()
    return nc


def kernel(feature, pointer_hidden_state, v, W):
    feature = np.ascontiguousarray(feature, dtype=np.float32)
    pointer_hidden_state = np.ascontiguousarray(pointer_hidden_state, dtype=np.float32)
    v = np.asarray(v, dtype=np.float32)
    W = np.asarray(W, dtype=np.float32)

    wfT = np.ascontiguousarray(W[0][:, :D].T)   # [e, d]
    whT = np.ascontiguousarray(W[0][:, D:].T)   # [e, d]
    vv = np.ascontiguousarray(v[0, 0][:, None])  # [D, 1]

    if "nc" not in _CACHED:
        _CACHED["nc"] = _build()
    nc = _CACHED["nc"]

    in_maps = []
    for c in range(N_CORES):
        sl = slice(c * B_PER, (c + 1) * B_PER)
        featT = np.ascontiguousarray(feature[sl].transpose(0, 2, 1))  # [B_PER, D, N]
        hT = np.ascontiguousarray(pointer_hidden_state[sl].T)         # [D, B_PER]
        in_maps.append({"featT": featT, "hT": hT, "wfT": wfT, "whT": whT, "vv": vv})

    res = run_bass_kernel_spmd(nc, in_maps, core_ids=list(range(N_CORES)))
    _CACHED["last_res"] = res
    outs = [res.results[c]["attns"] for c in range(N_CORES)]
    return np.concatenate(outs, axis=0)[:, None, :].astype(np.float32)
